# revision 37
# baseline (speedup 1.0000x reference)
"""ConvexSoftMixer Trainium2 kernel.

Shards batch*heads (1*8 = 8) across 8 NeuronCores, one head per core.

Math (exact refactor of the reference; m1 cancels analytically):
    f_q[s] = sum_j softplus(softplus(q @ spW1q.T + b1) @ spW2q.T + b2)[s,j]
    g_k[t] likewise for k
    phi_q = exp(q @ Wh.T); phi_k = exp(k @ Wh.T); u = v @ Wv.T
    c[t,p]  = g_k[t] + u[t,p]
    m2[p]   = max_t c[t,p]
    E[t,p]  = exp(c[t,p] - m2[p])
    M[r,p]  = sum_t phi_k[t,r] * E[t,p]
    y[s,p]  = f_q[s] + m2[p] + log( sum_r phi_q[s,r] * M[r,p] ) + delta
delta = sum(b2q) + sum(b2k) - log(S) is a pure additive output shift
(constant-in-t terms of g_k pass through max/exp/log unchanged), applied
on the host after gather.

Performance structure:
- All matmul operands are bf16 (1 PE cycle/row vs 4 for fp32; half the
  DMA bytes). PSUM accumulation stays fp32. |y| ~ 6e3 vs a 2e-2
  relative gate, so bf16 quantization (~3e-3 rel) is far inside budget.
- ONE consolidated input DMA (mega) for everything except vta: each
  dma_start's descriptor burst is staggered ~1.3us behind the previous
  on the DMA engines, so fewer dma_starts move the first matmul earlier.
- Layer-2 softplus is clamp-free: x + ln(1+exp(-x)) with -b2 folded
  into the Exp bias column; only the f/g row sums of layer 2 are needed,
  so the linear part comes from one matmul against host-precomputed
  column sums of spW2 and no z2 tensor is materialized. f_q/g_k land on
  PSUM partitions 0/32 (engine APs must start at 32-aligned partitions).
- All activations are Exp/Ln, forced onto the one ACT table holding
  both (natural_log_exp_and_others) so the 1283ns table load happens
  once, against a warm-up dummy activation that overlaps the input DMA.
- E-chunk accumulation matmuls run before the -m2 rank-1s so only the
  rank-1s wait on the reduce/transpose chain.
- The final y = ln(A) + f_q + m2 uses one rank-1 matmul for the f_q
  broadcast and a fused DVE scalar_tensor_tensor for the m2 column; the
  output DMA is triggered from the vector engine right after the add.
"""

import math

import numpy as np

_B, _H, _S, _D, _P = 1, 8, 512, 64, 32
_NCORES = 8
_LN_S = math.log(float(_S))

_CACHE = {}

# mega_a column map (bf16): what the z1 matmul needs — lands first
_XQK = 0
_W1 = 512
_B1 = 640
_MAW = 641
# mega_b column map (bf16): everything else — overlaps layer-1 compute
_W2 = 0
_WSUM = 128
_EQEK = 161
_WV = 194
_WH = 226
_NB2 = 258
_ONES = 259
_MBW = 387


def _build_bass(dump=False):
    import bass_rust as _bass_rust
    import concourse.tile as tile
    from concourse import bacc, mybir
    from concourse.alu_op_type import AluOpType
    from concourse.hw_specs import get_activation_tables

    f32 = mybir.dt.float32
    bf16 = mybir.dt.bfloat16
    AF = mybir.ActivationFunctionType
    AX = mybir.AxisListType.X

    nc = bacc.Bacc("TRN2", target_bir_lowering=False, debug=False)

    # All activations here are Exp/Ln; both live in the
    # natural_log_exp_and_others table. The stock ATL pass picks the
    # first table per function (exp->0, ln->5) and thrashes 1283ns
    # reloads on every switch. Hand it a table list (same names/order,
    # so emitted act_func_set_ids still index the real act_info.json)
    # where only the shared table advertises Exp/Ln.
    tabs = get_activation_tables(nc.m.arch)
    doctored = []
    for name, funcs in tabs.items():
        if name != "natural_log_exp_and_others":
            funcs = funcs - {AF.Exp, AF.Ln}
        doctored.append((name, funcs))
    nc.insert_act_table_loads = lambda: _bass_rust.insert_act_table_loads(
        nc, doctored
    )

    mega_a_d = nc.dram_tensor("mega_a", [128, _MAW], bf16,
                              kind="ExternalInput").ap()
    mega_b_d = nc.dram_tensor("mega_b", [128, _MBW], bf16,
                              kind="ExternalInput").ap()
    # vta rows: v (0:64) | g_k (64, device-written) | zeros (65:96) | ones
    # (96) — row 96 pairs with wv2's -m2 row, which must sit at a
    # 32-aligned partition for the engine write
    vta_d = nc.dram_tensor("vta", [97, _S], bf16, kind="ExternalInput").ap()
    y_d = nc.dram_tensor("y", [_P, _S], f32, kind="ExternalOutput").ap()

    NCH = _S // 128  # 4 sequence chunks of 128 for [t, p]-layout stages

    with tile.TileContext(nc) as tc:
        with (
            tc.tile_pool(name="pin", bufs=1) as pin,
            tc.tile_pool(name="pwork", bufs=1) as pw,
            tc.tile_pool(name="psA", bufs=2, space="PSUM") as psA,  # z1,z2/f,at
            tc.tile_pool(name="psB", bufs=2, space="PSUM") as psB,  # fg, cT
            tc.tile_pool(name="psC", bufs=2, space="PSUM") as psC,  # pk, ec
            tc.tile_pool(name="psD", bufs=1, space="PSUM") as psD,  # M
            tc.tile_pool(name="psE", bufs=1, space="PSUM") as psE,  # phiq
        ):
            # ---- input loads (z1's operands first), triggered from the
            # Activation engine: it reaches its first instruction ~2us
            # before the sync engine finishes its semaphore-init preamble
            mega = pin.tile([128, _MAW], bf16, tag="mega_a")
            nc.scalar.dma_start(out=mega, in_=mega_a_d)
            megb = pin.tile([128, _MBW], bf16, tag="mega_b")
            nc.scalar.dma_start(out=megb, in_=mega_b_d)
            vta = pin.tile([97, _S], bf16, tag="vta")
            nc.scalar.dma_start(out=vta, in_=vta_d)

            # ---- ACT table warm-up: no DMA deps, runs during input DMA ----
            scr = pw.tile([1, 2], f32, tag="scr")
            nc.vector.memset(scr, 0.0)
            scr2 = pw.tile([1, 2], f32, tag="scr2")
            nc.scalar.activation(out=scr2, in_=scr, func=AF.Exp, bias=0.0,
                                 scale=1.0)

            xqk = mega[:, _XQK:_XQK + _S]
            w1 = mega[:, _W1:_W1 + 128]   # block-diag softplus'd layer-1 (T)
            b1 = mega[:, _B1:_B1 + 1]     # stacked layer-1 bias column
            w2 = megb[:, _W2:_W2 + 128]
            wsum2 = megb[:, _WSUM:_WSUM + 33]  # col sums of spW2q / spW2k
            eqek = megb[:, _EQEK:_EQEK + 33]   # q-mask col 0, k-mask col 32
            wv_aug = megb[0:_D + 1, _WV:_WV + _P]  # Wv.T rows 0-63, row 64=1
            wh_t = megb[0:_D, _WH:_WH + _P]        # Wh.T (partitions 0-63)
            wh_tk = megb[_D:2 * _D, _WH:_WH + _P]  # Wh.T (partitions 64-127)
            negb2 = megb[:, _NB2:_NB2 + 1]  # -b2 column (layer-2 Exp bias)
            ones_row = megb[0:1, _ONES:_ONES + _P]   # [1,32] f_q broadcast

            # device-side Wv' for the E chunks: Wv.T | 1.0 (g_k) | zeros |
            # -m2 at row 96 (written after the reduce)
            wv2 = pw.tile([97, _P], bf16, tag="wv2")
            nc.vector.memset(wv2, 0.0)
            nc.vector.tensor_copy(out=wv2[0:_D + 1, :], in_=wv_aug)

            # ---- stacked ICNN layer 1 (q rows 0-63, k rows 64-127) ----
            z1_p = psA.tile([128, _S], f32, tag="big")
            nc.tensor.matmul(out=z1_p, lhsT=w1, rhs=xqk, start=True, stop=True)

            # phi_k chunks [t,r] / phi_q [r,s] fill the PE queue while the
            # scalar engine runs layer-1 softplus
            pk_p = psC.tile([128, NCH * _P], f32, tag="chunk")
            for c in range(NCH):
                nc.tensor.matmul(
                    out=pk_p[:, c * _P:(c + 1) * _P],
                    lhsT=xqk[_D:2 * _D, c * 128:(c + 1) * 128],
                    rhs=wh_tk,
                    start=True, stop=True,
                )
            phiq_p = psE.tile([_P, _S], f32, tag="phiq")
            nc.tensor.matmul(out=phiq_p, lhsT=wh_t, rhs=xqk[0:_D, :],
                             start=True, stop=True)

            # softplus layer 1: z1 = ln(1 + exp(z1_p + b1))
            e1 = pw.tile([128, _S], bf16, tag="e1")
            nc.scalar.activation(out=e1, in_=z1_p, func=AF.Exp, bias=b1,
                                 scale=1.0)
            z1 = pw.tile([128, _S], bf16, tag="z1")
            nc.scalar.activation(out=z1, in_=e1, func=AF.Ln, bias=1.0,
                                 scale=1.0)

            # ---- layer 2, clamp-free: z2 = x + ln(1+exp(-x)), x = pre+b2;
            # only the f/g row sums are needed downstream ----
            z2_p = psA.tile([128, _S], f32, tag="big")
            nc.tensor.matmul(out=z2_p, lhsT=w2, rhs=z1, start=True, stop=True)
            e2 = pw.tile([128, _S], bf16, tag="e2")
            nc.scalar.activation(out=e2, in_=z2_p, func=AF.Exp, bias=negb2,
                                 scale=-1.0)
            l2 = pw.tile([128, _S], bf16, tag="l2")
            nc.scalar.activation(out=l2, in_=e2, func=AF.Ln, bias=1.0,
                                 scale=1.0)

            # fg row 0 = f_q (sans sum(b2q)), row 32 = g_k (sans sum(b2k))
            fg_p = psB.tile([33, _S], f32, tag="mid")
            nc.tensor.matmul(out=fg_p, lhsT=wsum2, rhs=z1,
                             start=True, stop=False)
            nc.tensor.matmul(out=fg_p, lhsT=eqek, rhs=l2,
                             start=False, stop=True)

            # g_k into vta row 64 (pairs with wv_aug's 1.0 row); f_q copied
            # off the critical path for the late rank-1 broadcast
            nc.vector.tensor_copy(out=vta[_D:_D + 1, :],
                                  in_=fg_p[_P:_P + 1, :])
            fq = pw.tile([1, _S], bf16, tag="fq_sb")
            nc.vector.tensor_copy(out=fq, in_=fg_p[0:1, :])

            # ---- cT[p,t] = u.T + g_k broadcast ; m2 = rowmax ----
            cT_p = psB.tile([_P, _S], f32, tag="mid")
            nc.tensor.matmul(out=cT_p, lhsT=wv_aug, rhs=vta[0:_D + 1, :],
                             start=True, stop=True)
            m2pad = pw.tile([_P, _P], f32, tag="m2pad")
            nc.vector.memset(m2pad, 0.0)
            nc.vector.reduce_max(m2pad[:, 0:1], cT_p, axis=AX)
            m2t = pw.tile([_P, _P], f32, tag="m2t")
            nc.vector.transpose(m2t, m2pad)  # row 0 of m2t = m2 as [1, P]
            # -m2 into wv2 row 96 (32-aligned partition, bf16 cast on write)
            nc.vector.tensor_scalar_mul(wv2[96:97, :], m2t[0:1, 0:_P], -1.0)

            # ---- E chunks [t,p] = exp(u + g_k - m2), one matmul each ----
            ec_p = psC.tile([128, NCH * _P], f32, tag="chunk")
            for c in range(NCH):
                nc.tensor.matmul(
                    out=ec_p[:, c * _P:(c + 1) * _P],
                    lhsT=vta[:, c * 128:(c + 1) * 128],
                    rhs=wv2,
                    start=True, stop=True,
                )

            # scalar program order e1,z1ln,e2,l2,ec,pk,phiq,lnA: during e1
            # the 4-deep OOO window holds only the layer-1/2 chain (no
            # early-ready act can preempt z1ln), while pk/phiq still fill
            # the z2/ec wait gaps once z1ln has dispatched
            ec = pw.tile([128, NCH * _P], bf16, tag="ec")
            nc.scalar.activation(out=ec, in_=ec_p, func=AF.Exp, bias=0.0,
                                 scale=1.0)
            pk = pw.tile([128, NCH * _P], bf16, tag="pk")
            nc.scalar.activation(out=pk, in_=pk_p, func=AF.Exp, bias=0.0,
                                 scale=1.0)
            phiq = pw.tile([_P, _S], bf16, tag="phiq_sb")
            nc.scalar.activation(out=phiq, in_=phiq_p, func=AF.Exp, bias=0.0,
                                 scale=1.0)

            # ---- M[r,p] = sum_t phi_k E ----
            M_p = psD.tile([_P, _P], f32, tag="M")
            for c in range(NCH):
                nc.tensor.matmul(
                    out=M_p,
                    lhsT=pk[:, c * _P:(c + 1) * _P],
                    rhs=ec[:, c * _P:(c + 1) * _P],
                    start=(c == 0), stop=(c == NCH - 1),
                )
            M_sb = pw.tile([_P, _P], bf16, tag="M_sb")
            nc.vector.tensor_copy(out=M_sb, in_=M_p)

            # f_q broadcast [p,s] (rank-1); m2 joins in the final fused add
            f_p = psA.tile([_P, _S], f32, tag="big")
            nc.tensor.matmul(out=f_p, lhsT=ones_row, rhs=fq,
                             start=True, stop=True)

            at_p = psA.tile([_P, _S], f32, tag="big")
            nc.tensor.matmul(out=at_p, lhsT=M_sb, rhs=phiq,
                             start=True, stop=True)

            lnA = pw.tile([_P, _S], f32, tag="lnA")
            nc.scalar.activation(out=lnA, in_=at_p, func=AF.Ln, bias=0.0,
                                 scale=1.0)
            # y^T = (lnA + m2[p]) + F
            yT = pw.tile([_P, _S], f32, tag="yT")
            nc.vector.scalar_tensor_tensor(
                out=yT, in0=lnA, scalar=m2pad[:, 0:1], in1=f_p,
                op0=AluOpType.add, op1=AluOpType.add,
            )
            nc.sync.dma_start(out=y_d, in_=yT)

            if dump:
                for nm, t in [
                    ("d_z1", z1), ("d_l2", l2), ("d_fq", fq), ("d_pk", pk),
                    ("d_ec", ec), ("d_phiq", phiq), ("d_m2t", m2t),
                    ("d_Msb", M_sb), ("d_lnA", lnA),
                    ("d_vta64", vta[_D:_D + 1, :]), ("d_wv2", wv2),
                ]:
                    dd = nc.dram_tensor(nm, list(t.shape), t.dtype,
                                        kind="ExternalOutput").ap()
                    nc.sync.dma_start(out=dd, in_=t)

    if not nc.is_finalized():
        nc.finalize()
    return nc


def _host_inputs(q, k, v, spW1q, b1q, spW2q, b2q, spW1k, b1k, spW2k, b2k,
                 Wh, Wv):
    """Build the per-core input maps (numpy layout prep only)."""
    S, D, P = _S, _D, _P
    import ml_dtypes

    def b(x):
        return np.asarray(x, np.float32).astype(ml_dtypes.bfloat16)

    wa = np.zeros((128, _MAW - _S), np.float32)  # mega_a weight columns
    o = -_S
    wa[0:D, _W1 + o:_W1 + o + D] = spW1q.T
    wa[D:2 * D, _W1 + o + D:_W1 + o + 2 * D] = spW1k.T
    wa[0:D, _B1 + o] = b1q
    wa[D:2 * D, _B1 + o] = b1k
    wa_b = b(wa)

    mb = np.zeros((128, _MBW), np.float32)
    mb[0:D, _W2:_W2 + D] = spW2q.T
    mb[D:2 * D, _W2 + D:_W2 + 2 * D] = spW2k.T
    mb[0:D, _WSUM] = spW2q.sum(axis=0)   # wsumq[c] = sum_j spW2q[j,c]
    mb[D:2 * D, _WSUM + P] = spW2k.sum(axis=0)
    mb[0:D, _EQEK] = 1.0                 # eq
    mb[D:2 * D, _EQEK + P] = 1.0         # ek
    mb[0:D, _WV:_WV + P] = Wv.T
    mb[D, _WV:_WV + P] = 1.0             # pairs with the g_k row of vta
    mb[0:D, _WH:_WH + P] = Wh.T
    mb[D:2 * D, _WH:_WH + P] = Wh.T      # copy at base partition 64
    mb[0:D, _NB2] = -b2q
    mb[D:2 * D, _NB2] = -b2k
    mb[0, _ONES:_ONES + 128] = 1.0
    mb_b = b(mb)

    in_maps = []
    for h in range(_H):
        mega = np.zeros((128, _MAW), ml_dtypes.bfloat16)
        mega[0:D, 0:S] = b(q[0, h].T)
        mega[D:2 * D, 0:S] = b(k[0, h].T)
        mega[:, S:] = wa_b
        vta = np.zeros((97, S), np.float32)
        vta[0:D] = v[0, h].T
        # row D gets g_k on device; row 96 pairs with wv2's -m2 row
        vta[96] = 1.0
        in_maps.append(dict(mega_a=mega, mega_b=mb_b, vta=b(vta)))
    return in_maps


def kernel(**inputs):
    from concourse.bass_utils import run_bass_kernel_spmd

    np_in = {k: np.asarray(v) for k, v in inputs.items()}
    q, k, v = np_in["q"], np_in["k"], np_in["v"]

    def sp(x):  # softplus for the small weight matrices (host prep)
        return np.log1p(np.exp(x.astype(np.float64))).astype(np.float32)

    in_maps = _host_inputs(
        q, k, v,
        sp(np_in["sq_raw1"]), np_in["sq_b1"], sp(np_in["sq_raw2"]), np_in["sq_b2"],
        sp(np_in["sk_raw1"]), np_in["sk_b1"], sp(np_in["sk_raw2"]), np_in["sk_b2"],
        np_in["Wh"], np_in["Wv"],
    )
    # constant-in-t/s terms of y: sum(b2) linear parts and the -log(S)
    delta = (float(np_in["sq_b2"].sum()) + float(np_in["sk_b2"].sum())
             - _LN_S)

    if "nc" not in _CACHE:
        _CACHE["nc"] = _build_bass()
    nc = _CACHE["nc"]

    res = run_bass_kernel_spmd(nc, in_maps, list(range(_NCORES)))
    out = np.zeros((_B, _H, _S, _P), np.float32)
    for h in range(_H):
        out[0, h] = res.results[h]["y"].T + np.float32(delta)
    return out


# revision 41
# speedup vs baseline: 1.0629x; 1.0629x over previous
"""ConvexSoftMixer Trainium2 kernel.

Shards batch*heads (1*8 = 8) across 8 NeuronCores, one head per core.

Math (exact refactor of the reference; m1 cancels analytically):
    f_q[s] = sum_j softplus(softplus(q @ spW1q.T + b1) @ spW2q.T + b2)[s,j]
    g_k[t] likewise for k
    phi_q = exp(q @ Wh.T); phi_k = exp(k @ Wh.T); u = v @ Wv.T
    c[t,p]  = g_k[t] + u[t,p]
    m2[p]   = max_t c[t,p]
    E[t,p]  = exp(c[t,p] - m2[p])
    M[r,p]  = sum_t phi_k[t,r] * E[t,p]
    y[s,p]  = f_q[s] + m2[p] + log( sum_r phi_q[s,r] * M[r,p] ) + delta
delta = sum(b2q) + sum(b2k) - log(S) is a pure additive output shift
(constant-in-t terms of g_k pass through max/exp/log unchanged), applied
on the host after gather.

Performance structure:
- All matmul operands are bf16 (1 PE cycle/row vs 4 for fp32; half the
  DMA bytes). PSUM accumulation stays fp32. |y| ~ 6e3 vs a 2e-2
  relative gate, so bf16 quantization (~3e-3 rel) is far inside budget.
- ONE consolidated input DMA (mega) for everything except vta: each
  dma_start's descriptor burst is staggered ~1.3us behind the previous
  on the DMA engines, so fewer dma_starts move the first matmul earlier.
- Layer-2 softplus is clamp-free: x + ln(1+exp(-x)) with -b2 folded
  into the Exp bias column; only the f/g row sums of layer 2 are needed,
  so the linear part comes from one matmul against host-precomputed
  column sums of spW2 and no z2 tensor is materialized. f_q/g_k land on
  PSUM partitions 0/32 (engine APs must start at 32-aligned partitions).
- All activations are Exp/Ln, forced onto the one ACT table holding
  both (natural_log_exp_and_others) so the 1283ns table load happens
  once, against a warm-up dummy activation that overlaps the input DMA.
- E-chunk accumulation matmuls run before the -m2 rank-1s so only the
  rank-1s wait on the reduce/transpose chain.
- The final y = ln(A) + f_q + m2 uses one rank-1 matmul for the f_q
  broadcast and a fused DVE scalar_tensor_tensor for the m2 column; the
  output DMA is triggered from the vector engine right after the add.
"""

import math

import numpy as np

_B, _H, _S, _D, _P = 1, 8, 512, 64, 32
_NCORES = 8
_LN_S = math.log(float(_S))

_CACHE = {}

# mega_a column map (bf16): what the z1 matmul needs — lands first
_XQK = 0
_W1 = 512
_B1 = 640
_MAW = 641
# mega_b column map (bf16): everything else — overlaps layer-1 compute
_W2 = 0
_WSUM = 128
_EQEK = 161
_WV = 194
_WH = 226
_NB2 = 258
_ONES = 259
_MBW = 387


def _build_bass(dump=False):
    import bass_rust as _bass_rust
    import concourse.tile as tile
    from concourse import bacc, mybir
    from concourse.alu_op_type import AluOpType
    from concourse.hw_specs import get_activation_tables

    f32 = mybir.dt.float32
    bf16 = mybir.dt.bfloat16
    AF = mybir.ActivationFunctionType
    AX = mybir.AxisListType.X

    nc = bacc.Bacc("TRN2", target_bir_lowering=False, debug=False)

    # All activations here are Exp/Ln; both live in the
    # natural_log_exp_and_others table. The stock ATL pass picks the
    # first table per function (exp->0, ln->5) and thrashes 1283ns
    # reloads on every switch. Hand it a table list (same names/order,
    # so emitted act_func_set_ids still index the real act_info.json)
    # where only the shared table advertises Exp/Ln.
    tabs = get_activation_tables(nc.m.arch)
    doctored = []
    for name, funcs in tabs.items():
        if name != "natural_log_exp_and_others":
            funcs = funcs - {AF.Exp, AF.Ln}
        doctored.append((name, funcs))
    nc.insert_act_table_loads = lambda: _bass_rust.insert_act_table_loads(
        nc, doctored
    )

    mega_a_d = nc.dram_tensor("mega_a", [128, _MAW], bf16,
                              kind="ExternalInput").ap()
    mega_b_d = nc.dram_tensor("mega_b", [128, _MBW], bf16,
                              kind="ExternalInput").ap()
    # vta rows: v (0:64) | g_k (64, device-written) | zeros (65:96) | ones
    # (96) — row 96 pairs with wv2's -m2 row, which must sit at a
    # 32-aligned partition for the engine write. Only the v rows are
    # DMA'd; the constant rows are memset on device (every descriptor
    # saved matters: the DMA runtime dumps ~6x load on queue 0, whose
    # serial drain gates the g_k write via the tile WAW dependency).
    vta_d = nc.dram_tensor("vta", [_D, _S], bf16, kind="ExternalInput").ap()
    y_d = nc.dram_tensor("y", [_P, _S], f32, kind="ExternalOutput").ap()

    NCH = _S // 128  # 4 sequence chunks of 128 for [t, p]-layout stages

    with tile.TileContext(nc) as tc:
        with (
            tc.tile_pool(name="pin", bufs=1) as pin,
            tc.tile_pool(name="pwork", bufs=1) as pw,
            tc.tile_pool(name="psA", bufs=2, space="PSUM") as psA,  # z1,z2/f,at
            tc.tile_pool(name="psB", bufs=2, space="PSUM") as psB,  # fg, cT
            tc.tile_pool(name="psC", bufs=2, space="PSUM") as psC,  # pk, ec
            tc.tile_pool(name="psD", bufs=1, space="PSUM") as psD,  # M
            tc.tile_pool(name="psE", bufs=1, space="PSUM") as psE,  # phiq
        ):
            # ---- input loads (z1's operands first) ----
            mega = pin.tile([128, _MAW], bf16, tag="mega_a")
            nc.sync.dma_start(out=mega, in_=mega_a_d)
            megb = pin.tile([128, _MBW], bf16, tag="mega_b")
            nc.sync.dma_start(out=megb, in_=mega_b_d)
            vta = pin.tile([97, _S], bf16, tag="vta")
            nc.vector.memset(vta[_D:96, :], 0.0)
            nc.vector.memset(vta[96:97, :], 1.0)
            nc.sync.dma_start(out=vta[0:_D, :], in_=vta_d)

            # ---- ACT table warm-up: no DMA deps, runs during input DMA ----
            scr = pw.tile([1, 2], f32, tag="scr")
            nc.vector.memset(scr, 0.0)
            scr2 = pw.tile([1, 2], f32, tag="scr2")
            nc.scalar.activation(out=scr2, in_=scr, func=AF.Exp, bias=0.0,
                                 scale=1.0)

            xqk = mega[:, _XQK:_XQK + _S]
            w1 = mega[:, _W1:_W1 + 128]   # block-diag softplus'd layer-1 (T)
            b1 = mega[:, _B1:_B1 + 1]     # stacked layer-1 bias column
            w2 = megb[:, _W2:_W2 + 128]
            wsum2 = megb[:, _WSUM:_WSUM + 33]  # col sums of spW2q / spW2k
            eqek = megb[:, _EQEK:_EQEK + 33]   # q-mask col 0, k-mask col 32
            wv_aug = megb[0:_D + 1, _WV:_WV + _P]  # Wv.T rows 0-63, row 64=1
            wh_t = megb[0:_D, _WH:_WH + _P]        # Wh.T (partitions 0-63)
            wh_tk = megb[_D:2 * _D, _WH:_WH + _P]  # Wh.T (partitions 64-127)
            negb2 = megb[:, _NB2:_NB2 + 1]  # -b2 column (layer-2 Exp bias)
            ones_row = megb[0:1, _ONES:_ONES + _P]   # [1,32] f_q broadcast

            # device-side Wv' for the E chunks: Wv.T | 1.0 (g_k) | zeros |
            # -m2 at row 96 (written after the reduce)
            wv2 = pw.tile([97, _P], bf16, tag="wv2")
            nc.vector.memset(wv2, 0.0)
            nc.vector.tensor_copy(out=wv2[0:_D + 1, :], in_=wv_aug)

            # ---- stacked ICNN layer 1 (q rows 0-63, k rows 64-127) ----
            z1_p = psA.tile([128, _S], f32, tag="big")
            nc.tensor.matmul(out=z1_p, lhsT=w1, rhs=xqk, start=True, stop=True)

            # phi_k chunks [t,r] / phi_q [r,s] fill the PE queue while the
            # scalar engine runs layer-1 softplus
            pk_p = psC.tile([128, NCH * _P], f32, tag="chunk")
            for c in range(NCH):
                nc.tensor.matmul(
                    out=pk_p[:, c * _P:(c + 1) * _P],
                    lhsT=xqk[_D:2 * _D, c * 128:(c + 1) * 128],
                    rhs=wh_tk,
                    start=True, stop=True,
                )
            phiq_p = psE.tile([_P, _S], f32, tag="phiq")
            nc.tensor.matmul(out=phiq_p, lhsT=wh_t, rhs=xqk[0:_D, :],
                             start=True, stop=True)

            # softplus layer 1: z1 = ln(1 + exp(z1_p + b1))
            e1 = pw.tile([128, _S], bf16, tag="e1")
            nc.scalar.activation(out=e1, in_=z1_p, func=AF.Exp, bias=b1,
                                 scale=1.0)
            z1 = pw.tile([128, _S], bf16, tag="z1")
            nc.scalar.activation(out=z1, in_=e1, func=AF.Ln, bias=1.0,
                                 scale=1.0)

            # ---- layer 2, clamp-free: z2 = x + ln(1+exp(-x)), x = pre+b2;
            # only the f/g row sums are needed downstream ----
            z2_p = psA.tile([128, _S], f32, tag="big")
            nc.tensor.matmul(out=z2_p, lhsT=w2, rhs=z1, start=True, stop=True)
            e2 = pw.tile([128, _S], bf16, tag="e2")
            nc.scalar.activation(out=e2, in_=z2_p, func=AF.Exp, bias=negb2,
                                 scale=-1.0)
            l2 = pw.tile([128, _S], bf16, tag="l2")
            nc.scalar.activation(out=l2, in_=e2, func=AF.Ln, bias=1.0,
                                 scale=1.0)

            # fg row 0 = f_q (sans sum(b2q)), row 32 = g_k (sans sum(b2k))
            fg_p = psB.tile([33, _S], f32, tag="mid")
            nc.tensor.matmul(out=fg_p, lhsT=wsum2, rhs=z1,
                             start=True, stop=False)
            nc.tensor.matmul(out=fg_p, lhsT=eqek, rhs=l2,
                             start=False, stop=True)

            # g_k into vta row 64 (pairs with wv_aug's 1.0 row); f_q copied
            # off the critical path for the late rank-1 broadcast
            nc.vector.tensor_copy(out=vta[_D:_D + 1, :],
                                  in_=fg_p[_P:_P + 1, :])
            fq = pw.tile([1, _S], bf16, tag="fq_sb")
            nc.vector.tensor_copy(out=fq, in_=fg_p[0:1, :])

            # ---- cT[p,t] = u.T + g_k broadcast ; m2 = rowmax ----
            cT_p = psB.tile([_P, _S], f32, tag="mid")
            nc.tensor.matmul(out=cT_p, lhsT=wv_aug, rhs=vta[0:_D + 1, :],
                             start=True, stop=True)
            m2pad = pw.tile([_P, _P], f32, tag="m2pad")
            nc.vector.memset(m2pad, 0.0)
            nc.vector.reduce_max(m2pad[:, 0:1], cT_p, axis=AX)
            m2t = pw.tile([_P, _P], f32, tag="m2t")
            nc.vector.transpose(m2t, m2pad)  # row 0 of m2t = m2 as [1, P]
            # -m2 into wv2 row 96 (32-aligned partition, bf16 cast on write)
            nc.vector.tensor_scalar_mul(wv2[96:97, :], m2t[0:1, 0:_P], -1.0)

            # ---- E chunks [t,p] = exp(u + g_k - m2), one matmul each ----
            ec_p = psC.tile([128, NCH * _P], f32, tag="chunk")
            for c in range(NCH):
                nc.tensor.matmul(
                    out=ec_p[:, c * _P:(c + 1) * _P],
                    lhsT=vta[:, c * 128:(c + 1) * 128],
                    rhs=wv2,
                    start=True, stop=True,
                )

            # scalar program order e1,z1ln,e2,l2,ec,pk,phiq,lnA: during e1
            # the 4-deep OOO window holds only the layer-1/2 chain (no
            # early-ready act can preempt z1ln), while pk/phiq still fill
            # the z2/ec wait gaps once z1ln has dispatched
            ec = pw.tile([128, NCH * _P], bf16, tag="ec")
            nc.scalar.activation(out=ec, in_=ec_p, func=AF.Exp, bias=0.0,
                                 scale=1.0)
            pk = pw.tile([128, NCH * _P], bf16, tag="pk")
            nc.scalar.activation(out=pk, in_=pk_p, func=AF.Exp, bias=0.0,
                                 scale=1.0)
            phiq = pw.tile([_P, _S], bf16, tag="phiq_sb")
            nc.scalar.activation(out=phiq, in_=phiq_p, func=AF.Exp, bias=0.0,
                                 scale=1.0)

            # ---- M[r,p] = sum_t phi_k E ----
            M_p = psD.tile([_P, _P], f32, tag="M")
            for c in range(NCH):
                nc.tensor.matmul(
                    out=M_p,
                    lhsT=pk[:, c * _P:(c + 1) * _P],
                    rhs=ec[:, c * _P:(c + 1) * _P],
                    start=(c == 0), stop=(c == NCH - 1),
                )
            M_sb = pw.tile([_P, _P], bf16, tag="M_sb")
            nc.vector.tensor_copy(out=M_sb, in_=M_p)

            # f_q broadcast [p,s] (rank-1); m2 joins in the final fused add
            f_p = psA.tile([_P, _S], f32, tag="big")
            nc.tensor.matmul(out=f_p, lhsT=ones_row, rhs=fq,
                             start=True, stop=True)

            at_p = psA.tile([_P, _S], f32, tag="big")
            nc.tensor.matmul(out=at_p, lhsT=M_sb, rhs=phiq,
                             start=True, stop=True)

            lnA = pw.tile([_P, _S], f32, tag="lnA")
            nc.scalar.activation(out=lnA, in_=at_p, func=AF.Ln, bias=0.0,
                                 scale=1.0)
            # y^T = (lnA + m2[p]) + F
            yT = pw.tile([_P, _S], f32, tag="yT")
            nc.vector.scalar_tensor_tensor(
                out=yT, in0=lnA, scalar=m2pad[:, 0:1], in1=f_p,
                op0=AluOpType.add, op1=AluOpType.add,
            )
            nc.sync.dma_start(out=y_d, in_=yT)

            if dump:
                for nm, t in [
                    ("d_z1", z1), ("d_l2", l2), ("d_fq", fq), ("d_pk", pk),
                    ("d_ec", ec), ("d_phiq", phiq), ("d_m2t", m2t),
                    ("d_Msb", M_sb), ("d_lnA", lnA),
                    ("d_vta64", vta[_D:_D + 1, :]), ("d_wv2", wv2),
                ]:
                    dd = nc.dram_tensor(nm, list(t.shape), t.dtype,
                                        kind="ExternalOutput").ap()
                    nc.sync.dma_start(out=dd, in_=t)

    if not nc.is_finalized():
        nc.finalize()
    return nc


def _host_inputs(q, k, v, spW1q, b1q, spW2q, b2q, spW1k, b1k, spW2k, b2k,
                 Wh, Wv):
    """Build the per-core input maps (numpy layout prep only)."""
    S, D, P = _S, _D, _P
    import ml_dtypes

    def b(x):
        return np.asarray(x, np.float32).astype(ml_dtypes.bfloat16)

    wa = np.zeros((128, _MAW - _S), np.float32)  # mega_a weight columns
    o = -_S
    wa[0:D, _W1 + o:_W1 + o + D] = spW1q.T
    wa[D:2 * D, _W1 + o + D:_W1 + o + 2 * D] = spW1k.T
    wa[0:D, _B1 + o] = b1q
    wa[D:2 * D, _B1 + o] = b1k
    wa_b = b(wa)

    mb = np.zeros((128, _MBW), np.float32)
    mb[0:D, _W2:_W2 + D] = spW2q.T
    mb[D:2 * D, _W2 + D:_W2 + 2 * D] = spW2k.T
    mb[0:D, _WSUM] = spW2q.sum(axis=0)   # wsumq[c] = sum_j spW2q[j,c]
    mb[D:2 * D, _WSUM + P] = spW2k.sum(axis=0)
    mb[0:D, _EQEK] = 1.0                 # eq
    mb[D:2 * D, _EQEK + P] = 1.0         # ek
    mb[0:D, _WV:_WV + P] = Wv.T
    mb[D, _WV:_WV + P] = 1.0             # pairs with the g_k row of vta
    mb[0:D, _WH:_WH + P] = Wh.T
    mb[D:2 * D, _WH:_WH + P] = Wh.T      # copy at base partition 64
    mb[0:D, _NB2] = -b2q
    mb[D:2 * D, _NB2] = -b2k
    mb[0, _ONES:_ONES + 128] = 1.0
    mb_b = b(mb)

    in_maps = []
    for h in range(_H):
        mega = np.zeros((128, _MAW), ml_dtypes.bfloat16)
        mega[0:D, 0:S] = b(q[0, h].T)
        mega[D:2 * D, 0:S] = b(k[0, h].T)
        mega[:, S:] = wa_b
        in_maps.append(dict(mega_a=mega, mega_b=mb_b, vta=b(v[0, h].T)))
    return in_maps


def kernel(**inputs):
    from concourse.bass_utils import run_bass_kernel_spmd

    np_in = {k: np.asarray(v) for k, v in inputs.items()}
    q, k, v = np_in["q"], np_in["k"], np_in["v"]

    def sp(x):  # softplus for the small weight matrices (host prep)
        return np.log1p(np.exp(x.astype(np.float64))).astype(np.float32)

    in_maps = _host_inputs(
        q, k, v,
        sp(np_in["sq_raw1"]), np_in["sq_b1"], sp(np_in["sq_raw2"]), np_in["sq_b2"],
        sp(np_in["sk_raw1"]), np_in["sk_b1"], sp(np_in["sk_raw2"]), np_in["sk_b2"],
        np_in["Wh"], np_in["Wv"],
    )
    # constant-in-t/s terms of y: sum(b2) linear parts and the -log(S)
    delta = (float(np_in["sq_b2"].sum()) + float(np_in["sk_b2"].sum())
             - _LN_S)

    if "nc" not in _CACHE:
        _CACHE["nc"] = _build_bass()
    nc = _CACHE["nc"]

    res = run_bass_kernel_spmd(nc, in_maps, list(range(_NCORES)))
    out = np.zeros((_B, _H, _S, _P), np.float32)
    for h in range(_H):
        out[0, h] = res.results[h]["y"].T + np.float32(delta)
    return out
